# revision 23
# baseline (speedup 1.0000x reference)
"""Trainium2 Bass kernel for nn_AoAGNN (GATv2 GNN, 2 conv layers, attentional pooling).

Strategy (8 NeuronCores, SPMD):
  - Nodes partitioned into 8 contiguous ranges of 6250 (dst-range sharding).
  - Edges (incl. self loops) partitioned by dst core, sorted by dst, grouped
    into 128-dst-node blocks, padded to T tiles of 128 edges per block.
  - Per conv: node transforms computed per-shard; xl table AllGathered in bf16
    (8 shards of 6252 rows; last 2 rows of each shard are zeros used as safe
    dummy targets); xr table kept local.
  - Edge phase per block: three dma_gather ops (xl rows from the low/high
    halves of the gathered table - int16 index limit - plus xr rows from the
    local table), ulg = gA+gB, z = ulg+urg (DVE), leaky-relu (ACT),
    score = att-weighted reduce (DVE), exp (ACT), one-hot*weight matrices
    (DVE), segment aggregation + denominators (PE matmuls into PSUM).
  - Pooling partials [64, 129] per core; host sums partials and runs the tiny
    head MLP + final normalize.
"""
import numpy as np
import ml_dtypes

import concourse.bass as bass
import concourse.mybir as mybir
import concourse.tile as tile
from concourse import bacc
from concourse.bass_utils import run_bass_kernel_spmd

F32 = mybir.dt.float32
BF16 = mybir.dt.bfloat16
I16 = mybir.dt.int16
AF = mybir.ActivationFunctionType
ALU = mybir.AluOpType
AX = mybir.AxisListType

N, E, IN, HID, HEADS, G = 50000, 500000, 256, 128, 2, 64
SLOPE = 0.2
NC = 8
BLK = 128
C2 = HEADS * HID         # 256


def configure(n=50000, g=64, split=32768):
    """Set graph-size-derived globals (small values for sim tests)."""
    global N, G, NCN, NBLK, LASTM, SH, NTAB, SPLIT, ZA, ZB
    N, G = n, g
    NCN = N // NC
    NBLK = (NCN + BLK - 1) // BLK
    LASTM = NCN - (NBLK - 1) * BLK
    SH = NCN + 2
    NTAB = NC * SH
    SPLIT = min(split, NTAB)
    ZA = NCN
    if NTAB > SPLIT:
        s_ = min(s for s in range(NC) if s * SH + NCN >= SPLIT)
        ZB = s_ * SH + NCN - SPLIT
    else:
        ZB = 0


configure()

bf16 = ml_dtypes.bfloat16


def _bcast_mid(ap, t):
    """[128, F] AP -> [128, t, F] with step-0 middle dim."""
    (ps, pc), (fs, fc) = ap.ap
    return bass.AP(ap.tensor, ap.offset, [[ps, pc], [0, t], [fs, fc]])


def _wrap16(vals):
    """[K] index array -> [128, K//16] int16 in the dma_gather layout."""
    v = np.asarray(vals, np.int16).reshape(-1, 16).T        # [16, K/16]
    return np.tile(v, (8, 1))


# ----------------------------------------------------------------------------
# Host-side preprocessing
# ----------------------------------------------------------------------------

def host_prep(inputs):
    x = np.asarray(inputs['x'], np.float32)
    ei = np.asarray(inputs['edge_index'], np.int64)
    batch = np.asarray(inputs['batch'], np.int64)

    src = np.concatenate([ei[0], np.arange(N, dtype=np.int64)])
    dst = np.concatenate([ei[1], np.arange(N, dtype=np.int64)])

    core_of = dst // NCN
    per_core = []
    T = 0
    for c in range(NC):
        m = core_of == c
        s_c = src[m]
        d_c = dst[m] - c * NCN
        order = np.argsort(d_c, kind='stable')
        s_c, d_c = s_c[order], d_c[order]
        b_c = d_c // BLK
        starts = np.searchsorted(b_c, np.arange(NBLK + 1))
        per_core.append((s_c, d_c, starts))
        cnt = np.diff(starts)
        T = max(T, int(np.max((cnt + BLK - 1) // BLK)))

    SLOTS_B = T * BLK
    CI = NBLK * 24 * T     # EIDX16 cols: per block 3 gathers x 8T cols
    CM = NBLK * 2 * T      # emsk cols
    CS = NBLK * T          # eseg cols

    eidx_all, eseg_all, emsk_all, gseg_all, xT_all = [], [], [], [], []
    for c in range(NC):
        s_c, d_c, starts = per_core[c]
        eidx = np.zeros((BLK, CI), np.int16)
        eseg = np.zeros((BLK, CS), np.float32)
        emsk = np.zeros((BLK, CM), np.float32)
        for b in range(NBLK):
            lo, hi = starts[b], starts[b + 1]
            n_e = hi - lo
            rows = np.full(SLOTS_B, ZA, np.int64)
            sr = s_c[lo:hi]
            rows[:n_e] = (sr // NCN) * SH + (sr % NCN)
            ia = np.where(rows < SPLIT, rows, ZA).astype(np.int16)
            ib = np.where(rows >= SPLIT, rows - SPLIT, ZB).astype(np.int16)
            ir = np.full(SLOTS_B, 0, np.int16)
            ir[:n_e] = d_c[lo:hi]
            segs = np.zeros(SLOTS_B, np.float32)
            segs[:n_e] = d_c[lo:hi] - b * BLK
            msk = np.zeros(SLOTS_B, np.float32)
            msk[:n_e] = 1.0
            o = b * 24 * T
            eidx[:, o:o + 8 * T] = _wrap16(ia)
            eidx[:, o + 8 * T:o + 16 * T] = _wrap16(ib)
            eidx[:, o + 16 * T:o + 24 * T] = _wrap16(ir)
            eseg[:, b * T:(b + 1) * T] = segs.reshape(T, BLK).T
            emsk[:, b * 2 * T:(b + 1) * 2 * T] = np.repeat(
                msk.reshape(T, BLK).T, 2, axis=1)
        eidx_all.append(eidx)
        eseg_all.append(eseg)
        emsk_all.append(emsk)

        gseg = np.full((BLK, NBLK), 127.0, np.float32)
        bc = batch[c * NCN:(c + 1) * NCN].astype(np.float32)
        pad = np.full(NBLK * BLK - NCN, 127.0, np.float32)
        gseg[:, :] = np.concatenate([bc, pad]).reshape(NBLK, BLK).T
        gseg_all.append(gseg)

        xT_all.append(np.ascontiguousarray(x[c * NCN:(c + 1) * NCN].T).astype(bf16))

    return {'T': T, 'CI': CI, 'CM': CM, 'CS': CS,
            'eidx': eidx_all, 'eseg': eseg_all, 'emsk': emsk_all,
            'gseg': gseg_all, 'xT': xT_all}


def make_in_maps(inputs, host):
    inp = {k: np.asarray(v) for k, v in inputs.items()}
    shared = {}
    shared['iotaB'] = np.tile(np.arange(BLK, dtype=np.float32)[None, :], (BLK, 1)).astype(bf16)
    shared['giotaB'] = np.tile(np.arange(G, dtype=np.float32)[None, :], (BLK, 1)).astype(bf16)
    shared['identB'] = np.eye(BLK, dtype=np.float32).astype(bf16)
    shared['onesC'] = np.ones((BLK, 1), np.float32).astype(bf16)
    shared['enc_w1T'] = np.ascontiguousarray(inp['enc_w1'].T).astype(bf16)   # [256,128]
    shared['enc_w2T'] = np.ascontiguousarray(inp['enc_w2'].T).astype(bf16)   # [128,128]
    shared['enc_b1c'] = inp['enc_b1'].astype(np.float32).reshape(HID, 1)
    shared['enc_b2c'] = inp['enc_b2'].astype(np.float32).reshape(HID, 1)
    for p in ('c1', 'c2'):
        for side in ('l', 'r'):
            w = inp[f'{p}_w{side}']          # [256, 128]
            shared[f'{p}_w{side}T'] = np.ascontiguousarray(w.T).astype(bf16)  # [128,256]
            b = inp[f'{p}_b{side}'].astype(np.float32)
            shared[f'{p}_b{side}B'] = np.tile(b[None, :], (BLK, 1))           # [128,256]
        shared[f'{p}_attB'] = np.tile(
            np.asarray(inp[f'{p}_att']).ravel()[None, :], (BLK, 1)).astype(bf16)
        shared[f'{p}_biasB'] = np.tile(inp[f'{p}_bias'].astype(np.float32)[None, :], (BLK, 1))
    shared['gate_w1T'] = np.ascontiguousarray(inp['gate_w1'].T).astype(bf16)  # [128,128]
    shared['gate_b1B'] = np.tile(inp['gate_b1'].astype(np.float32)[None, :], (BLK, 1))
    shared['gate_w2B'] = np.tile(inp['gate_w2'].astype(np.float32), (BLK, 1)).astype(bf16)

    in_maps = []
    for c in range(NC):
        m = dict(shared)
        m['xT'] = host['xT'][c]
        m['eidx'] = host['eidx'][c]
        m['eseg'] = host['eseg'][c]
        m['emsk'] = host['emsk'][c]
        m['gseg'] = host['gseg'][c]
        in_maps.append(m)
    return in_maps


# ----------------------------------------------------------------------------
# Device program
# ----------------------------------------------------------------------------

def build_program(T, CI, CM, CS):
    nc = bacc.Bacc("TRN2", target_bir_lowering=False, debug=False,
                   enable_asserts=False, num_devices=NC)

    din = {}
    def ein(name, shape, dt):
        din[name] = nc.dram_tensor(name, list(shape), dt, kind="ExternalInput").ap()
        return din[name]

    ein('xT', (IN, NCN), BF16)
    ein('eidx', (BLK, CI), I16)
    ein('eseg', (BLK, CS), F32)
    ein('emsk', (BLK, CM), F32)
    ein('gseg', (BLK, NBLK), F32)
    ein('iotaB', (BLK, BLK), BF16)
    ein('giotaB', (BLK, G), BF16)
    ein('identB', (BLK, BLK), BF16)
    ein('onesC', (BLK, 1), BF16)
    ein('enc_w1T', (IN, HID), BF16)
    ein('enc_w2T', (HID, HID), BF16)
    ein('enc_b1c', (HID, 1), F32)
    ein('enc_b2c', (HID, 1), F32)
    for p in ('c1', 'c2'):
        ein(f'{p}_wlT', (HID, C2), BF16)
        ein(f'{p}_wrT', (HID, C2), BF16)
        ein(f'{p}_blB', (BLK, C2), F32)
        ein(f'{p}_brB', (BLK, C2), F32)
        ein(f'{p}_attB', (BLK, C2), BF16)
        ein(f'{p}_biasB', (BLK, HID), F32)
    ein('gate_w1T', (HID, HID), BF16)
    ein('gate_b1B', (BLK, HID), F32)
    ein('gate_w2B', (BLK, HID), BF16)

    pool_out = nc.dram_tensor("pool_out", [G, HID + 1], F32, kind="ExternalOutput").ap()

    RG = [list(range(NC))]

    from contextlib import ExitStack
    with tile.TileContext(nc) as tc, ExitStack() as stk:
        cst = stk.enter_context(tc.tile_pool(name="cst", bufs=1))
        sb = {}
        for k in ('iotaB', 'giotaB', 'identB', 'onesC', 'enc_w2T',
                  'enc_b1c', 'enc_b2c', 'gate_w1T', 'gate_b1B', 'gate_w2B',
                  'c1_wlT', 'c1_wrT', 'c1_blB', 'c1_brB', 'c1_attB', 'c1_biasB',
                  'c2_wlT', 'c2_wrT', 'c2_blB', 'c2_brB', 'c2_attB', 'c2_biasB',
                  'eidx', 'eseg', 'emsk', 'gseg'):
            ap = din[k]
            t = cst.tile(list(ap.shape), ap.dtype, name=f"sb_{k}")
            nc.sync.dma_start(t[:], ap)
            sb[k] = t
        for half in range(2):
            t = cst.tile([BLK, HID], BF16, name=f"sb_enc_w1T{half}")
            nc.sync.dma_start(t[:], din['enc_w1T'][half * BLK:(half + 1) * BLK, :])
            sb[f'enc_w1T{half}'] = t
        zrow = cst.tile([2, C2], BF16, name="zrow")
        nc.vector.memset(zrow[:], 0.0)

        NPAD = NBLK * BLK  # 6272
        hT_enc, _f1 = tc.tile([HID, NPAD], BF16, name="hT_enc")
        stk.callback(_f1)
        hT_c1, _f2 = tc.tile([HID, NPAD], BF16, name="hT_c1")
        stk.callback(_f2)

        dram = stk.enter_context(tc.tile_pool(name="dram", bufs=1, space="DRAM"))
        tabs, tabRs, shards = {}, {}, {}
        for p in ('c1', 'c2'):
            tabs[p] = dram.tile([NTAB, C2], BF16, name=f"tab_{p}", addr_space="Shared")
            tabRs[p] = dram.tile([NCN, C2], BF16, name=f"tabR_{p}")
            shards[p] = dram.tile([SH, C2], BF16, name=f"shard_{p}")

        # ------- encoder ---------------------------------------------------
        with tc.tile_pool(name="encp", bufs=2) as encp, \
             tc.tile_pool(name="encps", bufs=2, space="PSUM") as encps:
            xTs = []
            for half in range(2):
                xt = encp.tile([BLK, NCN], BF16, name=f"xT{half}", tag="xthalf")
                nc.sync.dma_start(xt[:], din['xT'][half * BLK:(half + 1) * BLK, :])
                xTs.append(xt)
            CH = 512
            nch = (NCN + CH - 1) // CH
            h1 = encp.tile([HID, NPAD], BF16, name="h1T", bufs=1)
            for i in range(nch):
                w = min(CH, NCN - i * CH)
                ps = encps.tile([HID, CH], F32, tag="encps")
                for half in range(2):
                    nc.tensor.matmul(ps[:, :w],
                                     sb[f'enc_w1T{half}'][:],
                                     xTs[half][:, i * CH:i * CH + w],
                                     start=(half == 0), stop=(half == 1))
                nc.scalar.activation(h1[:, i * CH:i * CH + w], ps[:, :w],
                                     AF.Relu, bias=sb['enc_b1c'][:])
            for i in range(nch):
                w = min(CH, NCN - i * CH)
                ps = encps.tile([HID, CH], F32, tag="encps")
                nc.tensor.matmul(ps[:, :w], sb['enc_w2T'][:],
                                 h1[:, i * CH:i * CH + w], start=True, stop=True)
                nc.scalar.activation(hT_enc[:, i * CH:i * CH + w], ps[:, :w],
                                     AF.Relu, bias=sb['enc_b2c'][:])

        # ------- per-conv --------------------------------------------------
        def conv(p, hT_in, hT_out, do_pool, edge=True):
            tab, tabR, shard = tabs[p], tabRs[p], shards[p]
            with tc.tile_pool(name=f"{p}tb", bufs=3) as tb, \
                 tc.tile_pool(name=f"{p}tbps", bufs=2, space="PSUM") as tbps:
                for nb in range(NBLK):
                    M = BLK if nb < NBLK - 1 else LASTM
                    lhs = hT_in[:, nb * BLK: nb * BLK + M]
                    for side, w_, b_ in (('l', f'{p}_wlT', f'{p}_blB'),
                                         ('r', f'{p}_wrT', f'{p}_brB')):
                        ps = tbps.tile([BLK, C2], F32, tag="tbps")
                        nc.tensor.matmul(ps[:M], lhs, sb[w_][:], start=True, stop=True)
                        tl = tb.tile([BLK, C2], BF16, tag="tbt")
                        nc.vector.tensor_tensor(out=tl[:M], in0=ps[:M],
                                                in1=sb[b_][:M], op=ALU.add)
                        if side == 'l':
                            nc.sync.dma_start(shard[nb * BLK: nb * BLK + M, :], tl[:M])
                        else:
                            nc.sync.dma_start(tabR[nb * BLK: nb * BLK + M, :], tl[:M])
                nc.sync.dma_start(shard[NCN:SH, :], zrow[:])
            nc.gpsimd.collective_compute(
                "AllGather", ALU.bypass, replica_groups=RG,
                ins=[shard[:].opt()], outs=[tab[:].opt()])

            if not edge:
                return
            attB = sb[f'{p}_attB']
            with tc.tile_pool(name=f"{p}eg", bufs=2) as eg, \
                 tc.tile_pool(name=f"{p}es", bufs=2) as es, \
                 tc.tile_pool(name=f"{p}ea", bufs=3) as ea, \
                 tc.tile_pool(name=f"{p}eps", bufs=2, space="PSUM") as eps, \
                 tc.tile_pool(name=f"{p}tps", bufs=2, space="PSUM") as tps, \
                 tc.tile_pool(name=f"{p}gps", bufs=1, space="PSUM") as gps:
                if do_pool:
                    poolps = gps.tile([G, HID + 1], F32, name="poolps", bufs=1)
                NI = T * BLK
                for b in range(NBLK):
                    io = b * 24 * T
                    mo = b * 2 * T
                    so = b * T
                    gA = eg.tile([BLK, T * C2], BF16, tag="gA")
                    gB = eg.tile([BLK, T * C2], BF16, tag="gB")
                    gR = eg.tile([BLK, T * C2], BF16, tag="gR")
                    GT = 8  # dma_gather num_idxs limit is 1024 = 8 tiles
                    for gbuf, src, xo in ((gA, tab[0:SPLIT, :], 0),
                                          (gB, tab[SPLIT:NTAB, :], 8 * T),
                                          (gR, tabR[:], 16 * T)):
                        for c0 in range(0, T, GT):
                            tl = min(GT, T - c0)
                            nc.gpsimd.dma_gather(
                                out_ap=gbuf[:, c0 * C2:(c0 + tl) * C2]
                                    .rearrange("p (t c) -> p t c", c=C2),
                                in_ap=src,
                                idxs_ap=sb['eidx'][:, io + xo + 8 * c0:
                                                   io + xo + 8 * (c0 + tl)],
                                num_idxs=tl * BLK, num_idxs_reg=tl * BLK,
                                elem_size=C2)
                    ulg = es.tile([BLK, T * C2], BF16, tag="ulg")
                    nc.vector.tensor_tensor(out=ulg[:], in0=gA[:], in1=gB[:], op=ALU.add)
                    z = es.tile([BLK, T * C2], BF16, tag="zblk")
                    nc.vector.tensor_tensor(out=z[:], in0=ulg[:], in1=gR[:], op=ALU.add)
                    lr = es.tile([BLK, T * C2], BF16, tag="lrblk")
                    nc.vector.scalar_tensor_tensor(out=lr[:], in0=z[:], scalar=SLOPE,
                                                   in1=z[:], op0=ALU.mult, op1=ALU.max)
                    lrat = es.tile([BLK, T * C2], BF16, tag="lrat")
                    nc.vector.tensor_tensor(out=lrat[:], in0=lr[:],
                                            in1=_bcast_mid(attB[:], T), op=ALU.mult)
                    esc = es.tile([BLK, 2 * T], F32, tag="esc")
                    nc.vector.tensor_reduce(
                        out=esc[:].rearrange("p (t h) -> p t h", h=HEADS),
                        in_=lrat[:].rearrange("p (t h x) -> p t h x", h=HEADS, x=HID),
                        axis=AX.X, op=ALU.add)
                    wexp = es.tile([BLK, 2 * T], F32, tag="wexp")
                    nc.scalar.activation(wexp[:], esc[:], AF.Exp)
                    wm = es.tile([BLK, 2 * T], F32, tag="wm")
                    nc.vector.tensor_tensor(out=wm[:], in0=wexp[:],
                                            in1=sb['emsk'][:, mo: mo + 2 * T],
                                            op=ALU.mult)
                    ps0 = eps.tile([BLK, HID], F32, tag="aggps")
                    ps1 = eps.tile([BLK, HID], F32, tag="aggps")
                    dn0 = eps.tile([BLK, 1], F32, tag="denps")
                    dn1 = eps.tile([BLK, 1], F32, tag="denps")
                    pss = (ps0, ps1)
                    dns = (dn0, dn1)
                    for t in range(T):
                        for h in range(HEADS):
                            A = ea.tile([BLK, BLK], BF16, tag="Ah")
                            nc.vector.tensor_scalar(
                                out=A[:], in0=sb['iotaB'][:],
                                scalar1=sb['eseg'][:, so + t: so + t + 1],
                                scalar2=wm[:, 2 * t + h: 2 * t + h + 1],
                                op0=ALU.is_equal, op1=ALU.mult)
                            nc.tensor.matmul(
                                pss[h][:, 0:HID], A[:],
                                ulg[:, t * C2 + h * HID: t * C2 + (h + 1) * HID],
                                start=(t == 0), stop=(t == T - 1))
                            nc.tensor.matmul(
                                dns[h][:, 0:1], A[:], sb['onesC'][:],
                                start=(t == 0), stop=(t == T - 1))
                    # epilogue
                    den = es.tile([BLK, 2], F32, tag="den")
                    for h in range(HEADS):
                        nc.vector.tensor_scalar(
                            out=den[:, h:h + 1], in0=dns[h][:, 0:1],
                            scalar1=1e-30, scalar2=None, op0=ALU.max)
                    inv = es.tile([BLK, 2], F32, tag="inv")
                    nc.vector.reciprocal(inv[:], den[:])
                    t0 = es.tile([BLK, HID], F32, tag="t0")
                    nc.vector.tensor_scalar(out=t0[:], in0=pss[0][:, 0:HID],
                                            scalar1=inv[:, 0:1], scalar2=None,
                                            op0=ALU.mult)
                    y = es.tile([BLK, HID], F32, tag="y")
                    nc.vector.scalar_tensor_tensor(out=y[:], in0=pss[1][:, 0:HID],
                                                   scalar=inv[:, 1:2], in1=t0[:],
                                                   op0=ALU.mult, op1=ALU.add)
                    yb = es.tile([BLK, HID], F32, tag="yb")
                    nc.vector.scalar_tensor_tensor(out=yb[:], in0=y[:], scalar=0.5,
                                                   in1=sb[f'{p}_biasB'][:],
                                                   op0=ALU.mult, op1=ALU.add)
                    hx = es.tile([BLK, HID + 1], BF16, tag="hx")
                    nc.scalar.activation(hx[:, 0:HID], yb[:], AF.Relu)
                    pt = tps.tile([BLK, BLK], BF16, tag="trps")
                    nc.tensor.transpose(pt[:], hx[:, 0:HID], sb['identB'][:])
                    nc.vector.tensor_copy(out=hT_out[:, b * BLK:(b + 1) * BLK],
                                          in_=pt[:])
                    import os as _os
                    PL = int(_os.environ.get('KPOOL', '9'))
                    if do_pool and PL >= 1:
                        nc.vector.memset(hx[:, HID:HID + 1], 1.0)
                        g1ps = gps.tile([BLK, HID], F32, tag="g1ps")
                        nc.tensor.matmul(g1ps[:], hT_out[:, b * BLK:(b + 1) * BLK],
                                         sb['gate_w1T'][:], start=True, stop=True)
                        if PL < 2:
                            continue
                        g1 = es.tile([BLK, HID], BF16, tag="g1")
                        nc.vector.tensor_tensor(out=g1[:], in0=g1ps[:],
                                                in1=sb['gate_b1B'][:], op=ALU.add)
                        g1r = es.tile([BLK, HID], BF16, tag="g1r")
                        nc.scalar.activation(g1r[:], g1[:], AF.Relu)
                        scr = es.tile([BLK, HID], F32, tag="scr")
                        gtb = es.tile([BLK, 1], F32, tag="gtb")
                        nc.vector.tensor_tensor(out=scr[:], in0=g1r[:],
                                                in1=sb['gate_w2B'][:], op=ALU.mult)
                        nc.vector.tensor_reduce(out=gtb[:], in_=scr[:],
                                                axis=AX.X, op=ALU.add)
                        if PL < 3:
                            continue
                        ge = es.tile([BLK, 1], F32, tag="ge")
                        nc.scalar.activation(ge[:], gtb[:], AF.Exp)
                        Ag = es.tile([BLK, G], BF16, tag="Ag")
                        nc.vector.tensor_scalar(
                            out=Ag[:], in0=sb['giotaB'][:],
                            scalar1=sb['gseg'][:, b:b + 1], scalar2=ge[:],
                            op0=ALU.is_equal, op1=ALU.mult)
                        if PL >= 4:
                            nc.tensor.matmul(poolps[:], Ag[:], hx[:],
                                             start=(b == 0), stop=(b == NBLK - 1))
                if do_pool:
                    po = es.tile([G, HID + 1], F32, name="po")
                    if int(__import__('os').environ.get('KPOOL', '9')) >= 4:
                        nc.vector.tensor_copy(out=po[:], in_=poolps[:])
                    else:
                        nc.vector.memset(po[:], 1.0)
                    nc.sync.dma_start(pool_out, po[:])

        import os
        stage = os.environ.get('KSTAGE', 'E')
        if stage in ('B', 'C', 'D', 'E'):
            conv('c1', hT_enc, hT_c1, do_pool=False, edge=stage != 'B')
        if stage in ('D', 'E'):
            conv('c2', hT_c1, hT_enc, do_pool=(stage == 'E'),
                 edge=(stage == 'E'))
        if stage != 'E':
            # dummy pool_out so the output tensor is always written
            with tc.tile_pool(name="dumo", bufs=1) as dp_:
                d_ = dp_.tile([G, HID + 1], F32, name="dummy_po")
                nc.vector.memset(d_[:], 1.0)
                nc.sync.dma_start(pool_out, d_[:])

    nc.compile()
    return nc


_CACHE = {}
LAST_RESULTS = None

def kernel(**inputs):
    host = host_prep(inputs)
    key = (host['T'], host['CI'], host['CM'], host['CS'])
    if key not in _CACHE:
        _CACHE[key] = build_program(*key)
    nc = _CACHE[key]
    in_maps = make_in_maps(inputs, host)
    import os
    trace = bool(int(os.environ.get('KTRACE', '0')))
    res = run_bass_kernel_spmd(nc, in_maps, core_ids=list(range(NC)),
                               trace=trace)
    global LAST_RESULTS
    LAST_RESULTS = res
    pool = np.zeros((G, HID + 1), np.float64)
    for r in res.results:
        pool += np.asarray(r['pool_out'], np.float64)
    g = (pool[:, :HID] / pool[:, HID:HID + 1]).astype(np.float32)
    out = (np.maximum(g @ np.asarray(inputs['head_w1'], np.float32).T
                      + np.asarray(inputs['head_b1'], np.float32), 0)
           @ np.asarray(inputs['head_w2'], np.float32).T
           + np.asarray(inputs['head_b2'], np.float32))
    out = out / np.maximum(np.linalg.norm(out, axis=1, keepdims=True), 1e-12)
    return out.astype(np.float32)
